# revision 29
# baseline (speedup 1.0000x reference)
"""GQA attention kernel for Trainium2, 8 NeuronCores.

Sharding: data-parallel over batch (4) x tensor-parallel over head groups (2).
Each core handles one (batch, head-group): 8 query heads / 2 kv heads.
o_proj is row-parallel -> host sums the 2 partial outputs per batch.

Layout strategy (per core):
  - Inputs host-prepped: xT = x[b].T (bf16), weight shards (bf16),
    RoPE tables cosT/sinT [128, T] (f32, sin sign-folded for rotate_half),
    128x128 lower-triangle mask.
  - Phase 1: QT[h] = (wq_h)^T x^T and KT[g] likewise (RoPE applied in
    [head_dim, T] layout; the 64-partition rotate-half shift is done via
    two small SBUF DMAs). V computed in natural [T, dh] layout.
  - Phase 2 (per 512-wide query tile, per head): S^T = K Q^T via
    lhsT=KT chunk, rhs=QT tile. Causal: streams are shortened on the four
    diagonal-crossing chunks (only valid q columns are computed) and a
    single 128x128 triangle mask multiply handles the partial block.
    exp on ScalarE over chunk PAIRS (one ACTIVATE covering 2 PSUM banks);
    S-matmul emission runs one pair ahead of the O/den matmuls so the PE
    never waits on the exp. O^T += V_chunk^T P^T; denominator via
    ones-vector matmul; 1/den via reciprocal_approx_fast; normalize O^T
    with a broadcast matmul of 1/den; o_proj from O^T with head-chunk
    outer loop and nt-paired PSUM tiles, f32 out.
"""

import json as _json

import numpy as np
import ml_dtypes

import concourse.bass as bass
import concourse.mybir as mybir
import concourse.tile as tile

# --- walrus sync-wait legalizer -------------------------------------------
# The walrus build in this container encodes at most ONE sync-wait command
# per instruction ("Too many sync wait commands" in setupSyncWait<> for any
# instruction with 2+ waits, including Tile's own tail Drain). Legalize by
# splitting extra waits into standalone single-wait EventSemaphore
# instructions on the same engine, immediately before the instruction —
# identical semantics (the engine stalls on each wait in turn).

_MAX_WAITS = 1
_orig_to_json_bytes = bass.Bass.to_json_bytes


def _split_waits_json(raw: bytes) -> bytes:
    m = _json.loads(raw)
    changed = False
    for fn in m.get("functions", []):
        for bb in fn.get("blocks", []):
            out = []
            for inst in bb.get("instructions", []):
                si = inst.get("sync_info")
                waits = (si or {}).get("on_wait") or []
                if len(waits) > _MAX_WAITS:
                    changed = True
                    for k, w in enumerate(waits[:-_MAX_WAITS]):
                        out.append({
                            "debug": inst.get("debug", 0),
                            "engine": inst["engine"],
                            "ins": [], "outs": [],
                            "name": f"{inst['name']}-sw{k}",
                            "opcode": "EventSemaphore",
                            "sync_info": {"on_update": [], "on_wait": [w]},
                        })
                    si["on_wait"] = waits[-_MAX_WAITS:]
                out.append(inst)
            bb["instructions"] = out
    if not changed:
        return raw
    return _json.dumps(m).encode()


def _patched_to_json_bytes(self):
    return _split_waits_json(_orig_to_json_bytes(self))


bass.Bass.to_json_bytes = _patched_to_json_bytes
# --------------------------------------------------------------------------

B, D = 4, 2048
NH, NKV, HD = 16, 4, 128
NHL, NKVL = 8, 2          # per-core q heads / kv heads
DQ = NHL * HD             # 1024
DKV = NKVL * HD           # 256
KD = D // 128             # 16 contraction chunks
TQ = 512                  # query tile width (matmul free dim)
THETA = 10000.0
SCALE = HD ** -0.5
NCORES = 8

bf16 = mybir.dt.bfloat16
f32 = mybir.dt.float32


def build_nc(T=2048, do_p1=True, do_p2=True):
    njq = T // TQ
    ts = bass.ts

    nc = bass.Bass()
    # All large inputs are host-prepped into partition-major contiguous
    # layouts so each load is ~128 fat descriptors instead of ~2048 strided
    # ones (strided descriptor-gen measured ~4.8us per load on the issuing
    # engine's queue).
    xTt = nc.dram_tensor("xTt", [T // TQ, 128, KD * TQ], bf16, kind="ExternalInput")
    wqa = nc.dram_tensor("wqa", [128, KD * (DQ // 2)], bf16, kind="ExternalInput")
    wqb = nc.dram_tensor("wqb", [128, KD * (DQ // 2)], bf16, kind="ExternalInput")
    wk = nc.dram_tensor("wk", [128, KD * DKV], bf16, kind="ExternalInput")
    wv = nc.dram_tensor("wv", [128, KD * DKV], bf16, kind="ExternalInput")
    wo = nc.dram_tensor("wo", [128, NHL * D], bf16, kind="ExternalInput")
    cosT = nc.dram_tensor("cosT", [HD, T], bf16, kind="ExternalInput")
    sinT = nc.dram_tensor("sinT", [HD, T], bf16, kind="ExternalInput")
    tri = nc.dram_tensor("tri", [128, 128], bf16, kind="ExternalInput")
    rot = nc.dram_tensor("rot", [128, 128], bf16, kind="ExternalInput")
    out = nc.dram_tensor("out", [T, D], f32, kind="ExternalOutput")

    with tile.TileContext(nc) as tc:
        with tc.tile_pool(name="res", bufs=1) as res, \
             tc.tile_pool(name="pp", bufs=3) as ppool, \
             tc.tile_pool(name="pot", bufs=2) as otpool:
            # ppool/otpool are top-level so phase-2 P and O^T tiles never
            # alias phase-1 SBUF (aliasing stalls the first phase-2 exps
            # behind the last phase-1 rope ops).
            QT_sb = res.tile([128, NHL, T], bf16)
            KT_sb = res.tile([128, NKVL, T], bf16)
            V_sb = res.tile([128, T // 128, DKV], bf16)
            tri_sb = res.tile([128, 128], bf16)
            rot_sb = res.tile([128, 128], bf16)
            ones_sb = res.tile([128, 1], bf16)
            onesr_sb = res.tile([1, 128], bf16)
            # wo lives in the persistent pool: its DMA streams during phase 1
            # with no SBUF-reuse wait (a w2-pool slot would alias the phase-1
            # weight region and stall the ACT queue at the phase boundary).
            wo_sb = res.tile([128, NHL, D], bf16)

            nc.vector.memset(ones_sb, 1.0)
            nc.vector.memset(onesr_sb, 1.0)
            if not do_p1:  # timing-attribution builds only
                nc.gpsimd.memset(QT_sb, 0.0)
                nc.gpsimd.memset(KT_sb, 0.0)
                nc.gpsimd.memset(V_sb, 0.0)

            # ---------------- Phase 1: projections + RoPE ----------------
            with tc.tile_pool(name="w1", bufs=1) as w1, \
                 tc.tile_pool(name="p1x", bufs=2) as xpool, \
                 tc.tile_pool(name="p1ps", bufs=2, space="PSUM") as pspool, \
                 tc.tile_pool(name="p1pv", bufs=3, space="PSUM") as pvpool, \
                 tc.tile_pool(name="p1rp", bufs=2, space="PSUM") as rppool, \
                 tc.tile_pool(name="p1t", bufs=2) as tpool:
                wqa_sb = w1.tile([128, KD, DQ // 2], bf16)
                wqb_sb = w1.tile([128, KD, DQ // 2], bf16)
                wk_sb = w1.tile([128, KD, DKV], bf16)
                wv_sb = w1.tile([128, KD, DKV], bf16)
                cos_sb = w1.tile([128, T], bf16)
                sin_sb = w1.tile([128, T], bf16)
                # Weights go over the Scalar HWDGE ring so they stream
                # concurrently with xt/cos/sin on the Sync ring (HWDGE DMAs
                # are FIFO per issuing engine). wv first: the per-tile
                # compute order below is V -> K -> Q, matching arrival.
                # Ring order matches PE consumption order (V -> K -> Q
                # heads 0-3 -> 4-7) with bytes balanced across the two HWDGE
                # rings; startup is HBM-bandwidth-bound so schedule is king.
                # sync: xt0(2M), wk(1M), wqb(2M), xt1... | scalar: wv(1M),
                # rot/cos/sin(1.1M), wqa(2M), wo(4M, last).
                nc.scalar.dma_start(out=wv_sb, in_=wv[:, :].rearrange("p (c m) -> p c m", c=KD))

                rope_pending = [None]
                for jt in range(njq if do_p1 else 0):
                    xt = xpool.tile([128, KD, TQ], bf16, tag="xt")
                    nc.sync.dma_start(out=xt, in_=xTt[jt].rearrange("p (c t) -> p c t", c=KD))
                    if jt == 0:
                        nc.sync.dma_start(out=wk_sb, in_=wk[:, :].rearrange("p (c m) -> p c m", c=KD))
                        nc.scalar.dma_start(out=rot_sb, in_=rot[:, :])
                        nc.scalar.dma_start(out=cos_sb, in_=cosT[:, :])
                        nc.scalar.dma_start(out=sin_sb, in_=sinT[:, :])
                        nc.scalar.dma_start(out=wqa_sb, in_=wqa[:, :].rearrange("p (c m) -> p c m", c=KD))
                        nc.sync.dma_start(out=wqb_sb, in_=wqb[:, :].rearrange("p (c m) -> p c m", c=KD))
                        nc.scalar.dma_start(out=wo_sb, in_=wo[:, :].rearrange("p (c n) -> p c n", c=NHL))
                    # V in natural [T, dkv] layout
                    for s in range(4):
                        pv = pvpool.tile([128, DKV], f32, tag="pv")
                        for c in range(KD):
                            nc.tensor.matmul(pv, lhsT=xt[:, c, s * 128:(s + 1) * 128],
                                             rhs=wv_sb[:, c, :],
                                             start=(c == 0), stop=(c == KD - 1))
                        nc.scalar.copy(V_sb[:, 4 * jt + s, :], pv)
                    # K then Q heads (transposed layout + RoPE).
                    # rotate_half is a PE matmul by a constant 64-shift
                    # permutation (rot_sb): no SBUF shift DMAs, no Sync-FIFO
                    # coupling. One-head lookahead: head h's rotate matmul is
                    # emitted after head h+1's projection matmuls so the PE
                    # never waits on the qf copy.
                    def emit_rope(pend):
                        hh, jt_, qf, ps = pend
                        if hh < NHL:
                            dst = QT_sb[:, hh, ts(jt_, TQ)]
                        else:
                            dst = KT_sb[:, hh - NHL, ts(jt_, TQ)]
                        rps = rppool.tile([128, TQ], f32, name="rps", tag="rps")
                        nc.tensor.matmul(rps, lhsT=rot_sb, rhs=qf,
                                         start=True, stop=True)
                        t1 = tpool.tile([128, TQ], bf16, tag="t1")
                        nc.vector.tensor_mul(t1, qf, cos_sb[:, ts(jt_, TQ)])
                        t2 = tpool.tile([128, TQ], bf16, tag="t2")
                        nc.vector.tensor_mul(t2, rps, sin_sb[:, ts(jt_, TQ)])
                        nc.vector.tensor_add(dst, t1, t2)

                    for h in [NHL, NHL + 1] + list(range(NHL)):
                        if h < NHL:
                            w_sb = wqa_sb if h < 4 else wqb_sb
                            col = (h % 4) * 128
                        else:
                            w_sb, col = wk_sb, (h - NHL) * 128
                        ps = pspool.tile([128, TQ], f32, tag="ps")
                        for c in range(KD):
                            nc.tensor.matmul(ps, lhsT=w_sb[:, c, col:col + 128],
                                             rhs=xt[:, c, :],
                                             start=(c == 0), stop=(c == KD - 1))
                        qf = tpool.tile([128, TQ], bf16, tag="qf")
                        nc.scalar.copy(qf, ps)
                        if rope_pending[0] is not None:
                            emit_rope(rope_pending[0])
                        rope_pending[0] = (h, jt, qf, ps)

                if do_p1 and rope_pending[0] is not None:
                    emit_rope(rope_pending[0])
                    rope_pending[0] = None

            # ---------------- Phase 2: attention + o_proj ----------------
            with tc.tile_pool(name="p2s", bufs=2, space="PSUM") as spool, \
                 tc.tile_pool(name="p2o", bufs=2, space="PSUM") as opool, \
                 tc.tile_pool(name="p2d", bufs=2, space="PSUM") as dpool, \
                 tc.tile_pool(name="p2t", bufs=2) as t2pool, \
                 tc.tile_pool(name="p2out", bufs=2) as outpool:
                nc.sync.dma_start(out=tri_sb, in_=tri[:, :])

                for jq in range(njq if do_p2 else 0):
                    OT = otpool.tile([128, NHL, TQ], bf16, tag="OT")
                    # per-head 1/den rows; distinct tags so all 8 stay live
                    # until the deferred normalize at the jq tail (matmul rhs
                    # must sit at base partition 0, so no shared [8,TQ] tile)
                    rden = {h: t2pool.tile([1, TQ], bf16, name=f"rden{h}",
                                           tag=f"rden{h}") for h in range(NHL)}

                    # Per head: chunk list with causal-shortened widths.
                    # chunk c < 4*jq: full (qoff=0, w=512);
                    # diagonal chunk c = 4*jq + r: qoff = r*128, w = 512-r*128.
                    nchunks = 4 * jq + 4

                    def chunk_geom(c):
                        r = c - 4 * jq
                        if r < 0:
                            return 0, TQ, False
                        return r * 128, TQ - r * 128, True

                    # pairs of chunks: (c0, c1) with c1 possibly absent
                    pairs = [(c0, c0 + 1 if c0 + 1 < nchunks else None)
                             for c0 in range(0, nchunks, 2)]

                    # flattened (h, pair) stream with 1-pair S-lookahead:
                    # S+exp+mask for pair i+1 are emitted before O/den
                    # matmuls of pair i, so the PE always has S work queued
                    # while the ACT exp for the previous pair completes.
                    stream = [(h, pi) for h in range(NHL) for pi in range(len(pairs))]
                    pending = []  # up to 2 of (h, pair, p_pair tile, geoms)
                    o_ps = {}
                    d_ps = {}

                    def emit_S(h, pi):
                        g = h // 4
                        c0, c1 = pairs[pi]
                        spair = spool.tile([128, 2 * TQ], f32, name="spair", tag="s")
                        geoms = []
                        off = 0
                        for c in (c0, c1):
                            if c is None:
                                continue
                            qoff, w, diag = chunk_geom(c)
                            nc.tensor.matmul(
                                spair[:, off:off + w],
                                lhsT=KT_sb[:, g, c * 128:(c + 1) * 128],
                                rhs=QT_sb[:, h, jq * TQ + qoff: (jq + 1) * TQ],
                                start=True, stop=True)
                            geoms.append((c, qoff, w, diag, off))
                            off += w
                        p_pair = ppool.tile([128, 2 * TQ], bf16, name="p_pair", tag="p")
                        nc.scalar.activation(p_pair[:, 0:off], spair[:, 0:off],
                                             mybir.ActivationFunctionType.Exp,
                                             scale=SCALE)
                        for (c, qoff, w, diag, off_c) in geoms:
                            if diag:
                                nc.vector.tensor_mul(
                                    p_pair[:, off_c:off_c + 128],
                                    p_pair[:, off_c:off_c + 128], tri_sb)
                        return (h, pi, p_pair, geoms)

                    def emit_O(pend):
                        h, pi, p_pair, geoms = pend
                        g = h // 4
                        for (c, qoff, w, diag, off_c) in geoms:
                            nc.tensor.matmul(
                                o_ps[h][:, qoff:TQ],
                                lhsT=V_sb[:, c, g * 128:(g + 1) * 128],
                                rhs=p_pair[:, off_c:off_c + w],
                                start=(c == 0), stop=(c == nchunks - 1))
                            nc.tensor.matmul(
                                d_ps[h][:, qoff:TQ],
                                lhsT=ones_sb, rhs=p_pair[:, off_c:off_c + w],
                                start=(c == 0), stop=(c == nchunks - 1))
                        if pi == len(pairs) - 1:
                            finish_head(h)

                    def finish_head(h):
                        # 1/den = exp(-ln(den)) on ScalarE (~1.1us vs 3.3us
                        # DVE iterative divide); Ln reads PSUM directly.
                        # Evacuate o_ps unnormalized and defer the broadcast/
                        # normalize to the jq tail so the PE never stalls on
                        # this chain mid-loop.
                        lden = t2pool.tile([1, TQ], f32, tag="lden")
                        nc.scalar.activation(lden, d_ps[h],
                                             mybir.ActivationFunctionType.Ln)
                        nc.scalar.activation(rden[h], lden,
                                             mybir.ActivationFunctionType.Exp,
                                             scale=-1.0)
                        nc.vector.tensor_copy(OT[:, h, :], o_ps[h])

                    for (h, pi) in stream:
                        if pi == 0:
                            o_ps[h] = opool.tile([128, TQ], f32, name="o_ps", tag="o")
                            d_ps[h] = dpool.tile([1, TQ], f32, name="d_ps", tag="d")
                        pending.append(emit_S(h, pi))
                        if len(pending) > 1:
                            emit_O(pending.pop(0))
                    while pending:
                        emit_O(pending.pop(0))

                    # deferred normalize: by now every head's rden row is
                    # long done, so the bc matmuls run back-to-back with no
                    # stall. bc tiles reuse the freed o_ps PSUM banks.
                    for h in range(NHL):
                        bc_ps = opool.tile([128, TQ], f32, name="bc_ps", tag="o")
                        nc.tensor.matmul(bc_ps, lhsT=onesr_sb,
                                         rhs=rden[h],
                                         start=True, stop=True)
                        bc_sb = t2pool.tile([128, TQ], bf16, tag="bc_sb")
                        nc.vector.tensor_copy(bc_sb, bc_ps)
                        nc.vector.tensor_mul(OT[:, h, :], OT[:, h, :], bc_sb)

                    # o_proj for this query tile (lhsT = O^T directly).
                    # head-chunk (hc) outer so the PE starts as soon as the
                    # first OT rows are normalized; nt-paired PSUM tiles so
                    # copies are [128, 1024].
                    # nt-pairs run sequentially: while pair A's [128,1024]
                    # copy drains, pair B's matmuls keep the PE fed, and the
                    # freed slot is ready for the next pair with no stall.
                    for s in range(4):
                        osb = outpool.tile([128, D], f32, tag="osb")
                        row = jq * TQ + s * 128
                        for half in range(2):
                            op = spool.tile([128, 2 * TQ], f32, name="op", tag="s")
                            for hc in range(NHL):
                                for nt in (2 * half, 2 * half + 1):
                                    nc.tensor.matmul(
                                        op[:, (nt % 2) * TQ:(nt % 2 + 1) * TQ],
                                        lhsT=OT[:, hc, s * 128:(s + 1) * 128],
                                        rhs=wo_sb[:, hc, ts(nt, TQ)],
                                        start=(hc == 0), stop=(hc == NHL - 1))
                            lo = half * 2 * TQ
                            nc.scalar.copy(osb[:, lo:lo + 2 * TQ], op)
                            nc.sync.dma_start(
                                out=out[row:row + 128, lo:lo + 2 * TQ],
                                in_=osb[:, lo:lo + 2 * TQ])
    return nc


def rope_tables(T=2048):
    inv = 1.0 / (THETA ** (np.arange(0, HD, 2, dtype=np.float32) / HD))
    t = np.arange(T, dtype=np.float32)
    freqs = np.outer(t, inv)
    emb = np.concatenate([freqs, freqs], -1)      # [T, 128]
    cos = np.ascontiguousarray(np.cos(emb).T.astype(ml_dtypes.bfloat16))
    sin = np.sin(emb).T.astype(np.float32)
    sin_signed = sin.copy()
    sin_signed[:64] *= -1.0                        # rotate_half sign fold
    return cos, np.ascontiguousarray(sin_signed.astype(ml_dtypes.bfloat16))


def tri_mask():
    k = np.arange(128)[:, None]
    q = np.arange(128)[None, :]
    return np.ascontiguousarray((k <= q).astype(ml_dtypes.bfloat16))


def rot_matrix():
    """lhsT for the rotate_half 64-shift: out = rotT.T @ q, out[i] = q[(i+64)%128]."""
    r = np.zeros((128, 128), dtype=ml_dtypes.bfloat16)
    for i in range(128):
        r[(i + 64) % 128, i] = 1.0
    return r


def _pmajor(w):
    """[D_chunks*128, M] -> [128, D_chunks*M] partition-major contiguous."""
    d, m = w.shape
    kd = d // 128
    return np.ascontiguousarray(
        w.reshape(kd, 128, m).transpose(1, 0, 2).reshape(128, kd * m))


def build_in_maps(x, wq, wk, wv, wo, T=2048):
    bf = ml_dtypes.bfloat16
    cos, sin_s = rope_tables(T)
    tri = tri_mask()
    rotm = rot_matrix()
    wq16 = np.asarray(wq).astype(bf)
    wk16 = np.asarray(wk).astype(bf)
    wv16 = np.asarray(wv).astype(bf)
    wo16 = np.asarray(wo).astype(bf)
    x = np.asarray(x)
    njq = T // TQ
    in_maps = []
    for core in range(NCORES):
        b, hg = core // 2, core % 2
        # xTt[jt] = [128, KD*TQ]: xTt[jt, p, c*TQ+t] = x[b][jt*TQ+t, c*128+p]
        xT = x[b].T.astype(bf).reshape(KD, 128, T)          # [c, p, t]
        xtt = np.stack([
            np.ascontiguousarray(
                xT[:, :, jt * TQ:(jt + 1) * TQ].transpose(1, 0, 2)
                .reshape(128, KD * TQ))
            for jt in range(njq)])
        wq_c = wq16[:, hg * DQ:(hg + 1) * DQ]
        in_maps.append({
            "xTt": xtt,
            "wqa": _pmajor(wq_c[:, 0:DQ // 2]),
            "wqb": _pmajor(wq_c[:, DQ // 2:DQ]),
            "wk": _pmajor(wk16[:, hg * DKV:(hg + 1) * DKV]),
            "wv": _pmajor(wv16[:, hg * DKV:(hg + 1) * DKV]),
            "wo": _pmajor(wo16[hg * DQ:(hg + 1) * DQ, :]),
            "cosT": cos, "sinT": sin_s, "tri": tri, "rot": rotm,
        })
    return in_maps


_NC_CACHE = {}


def get_nc(T=2048):
    if T not in _NC_CACHE:
        _NC_CACHE[T] = build_nc(T)
    return _NC_CACHE[T]


def run(inputs, trace=False, **kw):
    """Returns (full_output [B,T,D] f32, BassKernelResults)."""
    from concourse import bass_utils
    x = np.asarray(inputs["x"], dtype=np.float32)
    T = x.shape[1]
    nc = get_nc(T)
    in_maps = build_in_maps(x, inputs["wq"], inputs["wk"], inputs["wv"],
                            inputs["wo"], T)
    res = bass_utils.run_bass_kernel_spmd(nc, in_maps,
                                          core_ids=list(range(NCORES)),
                                          trace=trace, **kw)
    outs = [np.asarray(r["out"]) for r in res.results]
    full = np.empty((B, T, D), dtype=np.float32)
    for b in range(B):
        full[b] = outs[2 * b] + outs[2 * b + 1]
    return full, res


def kernel(x, mask, wq, wk, wv, wo):
    full, _ = run({"x": x, "mask": mask, "wq": wq, "wk": wk, "wv": wv, "wo": wo})
    return full


# revision 30
# speedup vs baseline: 1.0152x; 1.0152x over previous
"""GQA attention kernel for Trainium2, 8 NeuronCores.

Sharding: data-parallel over batch (4) x tensor-parallel over head groups (2).
Each core handles one (batch, head-group): 8 query heads / 2 kv heads.
o_proj is row-parallel -> host sums the 2 partial outputs per batch.

Layout strategy (per core):
  - Inputs host-prepped: xT = x[b].T (bf16), weight shards (bf16),
    RoPE tables cosT/sinT [128, T] (f32, sin sign-folded for rotate_half),
    128x128 lower-triangle mask.
  - Phase 1: QT[h] = (wq_h)^T x^T and KT[g] likewise (RoPE applied in
    [head_dim, T] layout; the 64-partition rotate-half shift is done via
    two small SBUF DMAs). V computed in natural [T, dh] layout.
  - Phase 2 (per 512-wide query tile, per head): S^T = K Q^T via
    lhsT=KT chunk, rhs=QT tile. Causal: streams are shortened on the four
    diagonal-crossing chunks (only valid q columns are computed) and a
    single 128x128 triangle mask multiply handles the partial block.
    exp on ScalarE over chunk PAIRS (one ACTIVATE covering 2 PSUM banks);
    S-matmul emission runs one pair ahead of the O/den matmuls so the PE
    never waits on the exp. O^T += V_chunk^T P^T; denominator via
    ones-vector matmul; 1/den via reciprocal_approx_fast; normalize O^T
    with a broadcast matmul of 1/den; o_proj from O^T with head-chunk
    outer loop and nt-paired PSUM tiles, f32 out.
"""

import json as _json

import numpy as np
import ml_dtypes

import concourse.bass as bass
import concourse.mybir as mybir
import concourse.tile as tile

# --- walrus sync-wait legalizer -------------------------------------------
# The walrus build in this container encodes at most ONE sync-wait command
# per instruction ("Too many sync wait commands" in setupSyncWait<> for any
# instruction with 2+ waits, including Tile's own tail Drain). Legalize by
# splitting extra waits into standalone single-wait EventSemaphore
# instructions on the same engine, immediately before the instruction —
# identical semantics (the engine stalls on each wait in turn).

_MAX_WAITS = 1
_orig_to_json_bytes = bass.Bass.to_json_bytes


def _split_waits_json(raw: bytes) -> bytes:
    m = _json.loads(raw)
    changed = False
    for fn in m.get("functions", []):
        for bb in fn.get("blocks", []):
            out = []
            for inst in bb.get("instructions", []):
                si = inst.get("sync_info")
                waits = (si or {}).get("on_wait") or []
                if len(waits) > _MAX_WAITS:
                    changed = True
                    for k, w in enumerate(waits[:-_MAX_WAITS]):
                        out.append({
                            "debug": inst.get("debug", 0),
                            "engine": inst["engine"],
                            "ins": [], "outs": [],
                            "name": f"{inst['name']}-sw{k}",
                            "opcode": "EventSemaphore",
                            "sync_info": {"on_update": [], "on_wait": [w]},
                        })
                    si["on_wait"] = waits[-_MAX_WAITS:]
                out.append(inst)
            bb["instructions"] = out
    if not changed:
        return raw
    return _json.dumps(m).encode()


def _patched_to_json_bytes(self):
    return _split_waits_json(_orig_to_json_bytes(self))


bass.Bass.to_json_bytes = _patched_to_json_bytes
# --------------------------------------------------------------------------

B, D = 4, 2048
NH, NKV, HD = 16, 4, 128
NHL, NKVL = 8, 2          # per-core q heads / kv heads
DQ = NHL * HD             # 1024
DKV = NKVL * HD           # 256
KD = D // 128             # 16 contraction chunks
TQ = 512                  # query tile width (matmul free dim)
THETA = 10000.0
SCALE = HD ** -0.5
NCORES = 8

bf16 = mybir.dt.bfloat16
f32 = mybir.dt.float32


def build_nc(T=2048, do_p1=True, do_p2=True):
    njq = T // TQ
    ts = bass.ts

    nc = bass.Bass()
    # All large inputs are host-prepped into partition-major contiguous
    # layouts so each load is ~128 fat descriptors instead of ~2048 strided
    # ones (strided descriptor-gen measured ~4.8us per load on the issuing
    # engine's queue).
    xTt = nc.dram_tensor("xTt", [T // TQ, 128, KD * TQ], bf16, kind="ExternalInput")
    wqa = nc.dram_tensor("wqa", [128, KD * (DQ // 2)], bf16, kind="ExternalInput")
    wqb = nc.dram_tensor("wqb", [128, KD * (DQ // 2)], bf16, kind="ExternalInput")
    wk = nc.dram_tensor("wk", [128, KD * DKV], bf16, kind="ExternalInput")
    wv = nc.dram_tensor("wv", [128, KD * DKV], bf16, kind="ExternalInput")
    wo = nc.dram_tensor("wo", [128, NHL * D], bf16, kind="ExternalInput")
    cosT = nc.dram_tensor("cosT", [HD, T], bf16, kind="ExternalInput")
    sinT = nc.dram_tensor("sinT", [HD, T], bf16, kind="ExternalInput")
    tri = nc.dram_tensor("tri", [128, 128], bf16, kind="ExternalInput")
    out = nc.dram_tensor("out", [T, D], f32, kind="ExternalOutput")

    with tile.TileContext(nc) as tc:
        with tc.tile_pool(name="res", bufs=1) as res, \
             tc.tile_pool(name="pp", bufs=3) as ppool, \
             tc.tile_pool(name="pot", bufs=2) as otpool:
            # ppool/otpool are top-level so phase-2 P and O^T tiles never
            # alias phase-1 SBUF (aliasing stalls the first phase-2 exps
            # behind the last phase-1 rope ops).
            QT_sb = res.tile([128, NHL, T], bf16)
            KT_sb = res.tile([128, NKVL, T], bf16)
            V_sb = res.tile([128, T // 128, DKV], bf16)
            tri_sb = res.tile([128, 128], bf16)
            ones_sb = res.tile([128, 1], bf16)
            onesr_sb = res.tile([1, 128], bf16)
            # wo lives in the persistent pool: its DMA streams during phase 1
            # with no SBUF-reuse wait (a w2-pool slot would alias the phase-1
            # weight region and stall the ACT queue at the phase boundary).
            wo_sb = res.tile([128, NHL, D], bf16)

            nc.vector.memset(ones_sb, 1.0)
            nc.vector.memset(onesr_sb, 1.0)
            if not do_p1:  # timing-attribution builds only
                nc.gpsimd.memset(QT_sb, 0.0)
                nc.gpsimd.memset(KT_sb, 0.0)
                nc.gpsimd.memset(V_sb, 0.0)

            # ---------------- Phase 1: projections + RoPE ----------------
            with tc.tile_pool(name="w1", bufs=1) as w1, \
                 tc.tile_pool(name="p1x", bufs=2) as xpool, \
                 tc.tile_pool(name="p1ps", bufs=2, space="PSUM") as pspool, \
                 tc.tile_pool(name="p1pv", bufs=3, space="PSUM") as pvpool, \
                 tc.tile_pool(name="p1t", bufs=3) as tpool:
                wqa_sb = w1.tile([128, KD, DQ // 2], bf16)
                wqb_sb = w1.tile([128, KD, DQ // 2], bf16)
                wk_sb = w1.tile([128, KD, DKV], bf16)
                wv_sb = w1.tile([128, KD, DKV], bf16)
                cos_sb = w1.tile([128, T], bf16)
                sin_sb = w1.tile([128, T], bf16)
                # Weights go over the Scalar HWDGE ring so they stream
                # concurrently with xt/cos/sin on the Sync ring (HWDGE DMAs
                # are FIFO per issuing engine). wv first: the per-tile
                # compute order below is V -> K -> Q, matching arrival.
                # Ring order matches PE consumption order (V -> K -> Q
                # heads 0-3 -> 4-7) with bytes balanced across the two HWDGE
                # rings; startup is HBM-bandwidth-bound so schedule is king.
                # sync: xt0(2M), wk(1M), wqb(2M), xt1... | scalar: wv(1M),
                # rot/cos/sin(1.1M), wqa(2M), wo(4M, last).
                nc.scalar.dma_start(out=wv_sb, in_=wv[:, :].rearrange("p (c m) -> p c m", c=KD))

                for jt in range(njq if do_p1 else 0):
                    xt = xpool.tile([128, KD, TQ], bf16, tag="xt")
                    nc.sync.dma_start(out=xt, in_=xTt[jt].rearrange("p (c t) -> p c t", c=KD))
                    if jt == 0:
                        nc.sync.dma_start(out=wk_sb, in_=wk[:, :].rearrange("p (c m) -> p c m", c=KD))
                        nc.scalar.dma_start(out=cos_sb, in_=cosT[:, :])
                        nc.scalar.dma_start(out=sin_sb, in_=sinT[:, :])
                        nc.scalar.dma_start(out=wqa_sb, in_=wqa[:, :].rearrange("p (c m) -> p c m", c=KD))
                        nc.scalar.dma_start(out=wqb_sb, in_=wqb[:, :].rearrange("p (c m) -> p c m", c=KD))
                        nc.scalar.dma_start(out=wo_sb, in_=wo[:, :].rearrange("p (c n) -> p c n", c=NHL))
                    # V in natural [T, dkv] layout
                    for s in range(4):
                        pv = pvpool.tile([128, DKV], f32, tag="pv")
                        for c in range(KD):
                            nc.tensor.matmul(pv, lhsT=xt[:, c, s * 128:(s + 1) * 128],
                                             rhs=wv_sb[:, c, :],
                                             start=(c == 0), stop=(c == KD - 1))
                        nc.scalar.copy(V_sb[:, 4 * jt + s, :], pv)
                    # K then Q heads (transposed layout + RoPE in bf16:
                    # the rotate_half 64-partition shift via two small SBUF
                    # DMAs on the Sync ring, which carries only xt loads
                    # besides these).
                    for h in [NHL, NHL + 1] + list(range(NHL)):
                        if h < NHL:
                            w_sb = wqa_sb if h < 4 else wqb_sb
                            col = (h % 4) * 128
                            dst = QT_sb[:, h, ts(jt, TQ)]
                        else:
                            w_sb, col = wk_sb, (h - NHL) * 128
                            dst = KT_sb[:, h - NHL, ts(jt, TQ)]
                        ps = pspool.tile([128, TQ], f32, tag="ps")
                        for c in range(KD):
                            nc.tensor.matmul(ps, lhsT=w_sb[:, c, col:col + 128],
                                             rhs=xt[:, c, :],
                                             start=(c == 0), stop=(c == KD - 1))
                        qf = tpool.tile([128, TQ], bf16, tag="qf")
                        nc.scalar.copy(qf, ps)
                        qs = tpool.tile([128, TQ], bf16, tag="qs")
                        nc.sync.dma_start(out=qs[0:64, :], in_=qf[64:128, :])
                        nc.sync.dma_start(out=qs[64:128, :], in_=qf[0:64, :])
                        t1 = tpool.tile([128, TQ], bf16, tag="t1")
                        nc.vector.tensor_mul(t1, qf, cos_sb[:, ts(jt, TQ)])
                        nc.vector.tensor_mul(qs, qs, sin_sb[:, ts(jt, TQ)])
                        nc.vector.tensor_add(dst, t1, qs)

            # ---------------- Phase 2: attention + o_proj ----------------
            with tc.tile_pool(name="p2s", bufs=2, space="PSUM") as spool, \
                 tc.tile_pool(name="p2o", bufs=2, space="PSUM") as opool, \
                 tc.tile_pool(name="p2d", bufs=2, space="PSUM") as dpool, \
                 tc.tile_pool(name="p2t", bufs=2) as t2pool, \
                 tc.tile_pool(name="p2out", bufs=2) as outpool:
                nc.sync.dma_start(out=tri_sb, in_=tri[:, :])

                for jq in range(njq if do_p2 else 0):
                    OT = otpool.tile([128, NHL, TQ], bf16, tag="OT")
                    # per-head 1/den rows; distinct tags so all 8 stay live
                    # until the deferred normalize at the jq tail (matmul rhs
                    # must sit at base partition 0, so no shared [8,TQ] tile)
                    rden = {h: t2pool.tile([1, TQ], bf16, name=f"rden{h}",
                                           tag=f"rden{h}") for h in range(NHL)}

                    # Per head: chunk list with causal-shortened widths.
                    # chunk c < 4*jq: full (qoff=0, w=512);
                    # diagonal chunk c = 4*jq + r: qoff = r*128, w = 512-r*128.
                    nchunks = 4 * jq + 4

                    def chunk_geom(c):
                        r = c - 4 * jq
                        if r < 0:
                            return 0, TQ, False
                        return r * 128, TQ - r * 128, True

                    # pairs of chunks: (c0, c1) with c1 possibly absent
                    pairs = [(c0, c0 + 1 if c0 + 1 < nchunks else None)
                             for c0 in range(0, nchunks, 2)]

                    # flattened (h, pair) stream with 1-pair S-lookahead:
                    # S+exp+mask for pair i+1 are emitted before O/den
                    # matmuls of pair i, so the PE always has S work queued
                    # while the ACT exp for the previous pair completes.
                    stream = [(h, pi) for h in range(NHL) for pi in range(len(pairs))]
                    pending = []  # up to 2 of (h, pair, p_pair tile, geoms)
                    o_ps = {}
                    d_ps = {}

                    def emit_S(h, pi):
                        g = h // 4
                        c0, c1 = pairs[pi]
                        spair = spool.tile([128, 2 * TQ], f32, name="spair", tag="s")
                        geoms = []
                        off = 0
                        for c in (c0, c1):
                            if c is None:
                                continue
                            qoff, w, diag = chunk_geom(c)
                            nc.tensor.matmul(
                                spair[:, off:off + w],
                                lhsT=KT_sb[:, g, c * 128:(c + 1) * 128],
                                rhs=QT_sb[:, h, jq * TQ + qoff: (jq + 1) * TQ],
                                start=True, stop=True)
                            geoms.append((c, qoff, w, diag, off))
                            off += w
                        p_pair = ppool.tile([128, 2 * TQ], bf16, name="p_pair", tag="p")
                        nc.scalar.activation(p_pair[:, 0:off], spair[:, 0:off],
                                             mybir.ActivationFunctionType.Exp,
                                             scale=SCALE)
                        for (c, qoff, w, diag, off_c) in geoms:
                            if diag:
                                nc.vector.tensor_mul(
                                    p_pair[:, off_c:off_c + 128],
                                    p_pair[:, off_c:off_c + 128], tri_sb)
                        return (h, pi, p_pair, geoms)

                    def emit_O(pend):
                        h, pi, p_pair, geoms = pend
                        g = h // 4
                        for (c, qoff, w, diag, off_c) in geoms:
                            nc.tensor.matmul(
                                o_ps[h][:, qoff:TQ],
                                lhsT=V_sb[:, c, g * 128:(g + 1) * 128],
                                rhs=p_pair[:, off_c:off_c + w],
                                start=(c == 0), stop=(c == nchunks - 1))
                            nc.tensor.matmul(
                                d_ps[h][:, qoff:TQ],
                                lhsT=ones_sb, rhs=p_pair[:, off_c:off_c + w],
                                start=(c == 0), stop=(c == nchunks - 1))
                        if pi == len(pairs) - 1:
                            finish_head(h)

                    def finish_head(h):
                        # 1/den = exp(-ln(den)) on ScalarE (~1.1us vs 3.3us
                        # DVE iterative divide); Ln reads PSUM directly.
                        # Evacuate o_ps unnormalized and defer the broadcast/
                        # normalize to the jq tail so the PE never stalls on
                        # this chain mid-loop.
                        lden = t2pool.tile([1, TQ], f32, tag="lden")
                        nc.scalar.activation(lden, d_ps[h],
                                             mybir.ActivationFunctionType.Ln)
                        nc.scalar.activation(rden[h], lden,
                                             mybir.ActivationFunctionType.Exp,
                                             scale=-1.0)
                        nc.vector.tensor_copy(OT[:, h, :], o_ps[h])

                    for (h, pi) in stream:
                        if pi == 0:
                            o_ps[h] = opool.tile([128, TQ], f32, name="o_ps", tag="o")
                            d_ps[h] = dpool.tile([1, TQ], f32, name="d_ps", tag="d")
                        pending.append(emit_S(h, pi))
                        if len(pending) > 1:
                            emit_O(pending.pop(0))
                    while pending:
                        emit_O(pending.pop(0))

                    # deferred normalize: by now every head's rden row is
                    # long done, so the bc matmuls run back-to-back with no
                    # stall. bc tiles reuse the freed o_ps PSUM banks.
                    for h in range(NHL):
                        bc_ps = opool.tile([128, TQ], f32, name="bc_ps", tag="o")
                        nc.tensor.matmul(bc_ps, lhsT=onesr_sb,
                                         rhs=rden[h],
                                         start=True, stop=True)
                        bc_sb = t2pool.tile([128, TQ], bf16, tag="bc_sb")
                        nc.vector.tensor_copy(bc_sb, bc_ps)
                        nc.vector.tensor_mul(OT[:, h, :], OT[:, h, :], bc_sb)

                    # o_proj for this query tile (lhsT = O^T directly).
                    # head-chunk (hc) outer so the PE starts as soon as the
                    # first OT rows are normalized; nt-paired PSUM tiles so
                    # copies are [128, 1024].
                    # nt-pairs run sequentially: while pair A's [128,1024]
                    # copy drains, pair B's matmuls keep the PE fed, and the
                    # freed slot is ready for the next pair with no stall.
                    for s in range(4):
                        osb = outpool.tile([128, D], f32, tag="osb")
                        row = jq * TQ + s * 128
                        for half in range(2):
                            op = spool.tile([128, 2 * TQ], f32, name="op", tag="s")
                            for hc in range(NHL):
                                for nt in (2 * half, 2 * half + 1):
                                    nc.tensor.matmul(
                                        op[:, (nt % 2) * TQ:(nt % 2 + 1) * TQ],
                                        lhsT=OT[:, hc, s * 128:(s + 1) * 128],
                                        rhs=wo_sb[:, hc, ts(nt, TQ)],
                                        start=(hc == 0), stop=(hc == NHL - 1))
                            lo = half * 2 * TQ
                            nc.scalar.copy(osb[:, lo:lo + 2 * TQ], op)
                            nc.sync.dma_start(
                                out=out[row:row + 128, lo:lo + 2 * TQ],
                                in_=osb[:, lo:lo + 2 * TQ])
    return nc


def rope_tables(T=2048):
    inv = 1.0 / (THETA ** (np.arange(0, HD, 2, dtype=np.float32) / HD))
    t = np.arange(T, dtype=np.float32)
    freqs = np.outer(t, inv)
    emb = np.concatenate([freqs, freqs], -1)      # [T, 128]
    cos = np.ascontiguousarray(np.cos(emb).T.astype(ml_dtypes.bfloat16))
    sin = np.sin(emb).T.astype(np.float32)
    sin_signed = sin.copy()
    sin_signed[:64] *= -1.0                        # rotate_half sign fold
    return cos, np.ascontiguousarray(sin_signed.astype(ml_dtypes.bfloat16))


def tri_mask():
    k = np.arange(128)[:, None]
    q = np.arange(128)[None, :]
    return np.ascontiguousarray((k <= q).astype(ml_dtypes.bfloat16))


def rot_matrix():
    """lhsT for the rotate_half 64-shift: out = rotT.T @ q, out[i] = q[(i+64)%128]."""
    r = np.zeros((128, 128), dtype=ml_dtypes.bfloat16)
    for i in range(128):
        r[(i + 64) % 128, i] = 1.0
    return r


def _pmajor(w):
    """[D_chunks*128, M] -> [128, D_chunks*M] partition-major contiguous."""
    d, m = w.shape
    kd = d // 128
    return np.ascontiguousarray(
        w.reshape(kd, 128, m).transpose(1, 0, 2).reshape(128, kd * m))


def build_in_maps(x, wq, wk, wv, wo, T=2048):
    bf = ml_dtypes.bfloat16
    cos, sin_s = rope_tables(T)
    tri = tri_mask()
    wq16 = np.asarray(wq).astype(bf)
    wk16 = np.asarray(wk).astype(bf)
    wv16 = np.asarray(wv).astype(bf)
    wo16 = np.asarray(wo).astype(bf)
    x = np.asarray(x)
    njq = T // TQ
    in_maps = []
    for core in range(NCORES):
        b, hg = core // 2, core % 2
        # xTt[jt] = [128, KD*TQ]: xTt[jt, p, c*TQ+t] = x[b][jt*TQ+t, c*128+p]
        xT = x[b].T.astype(bf).reshape(KD, 128, T)          # [c, p, t]
        xtt = np.stack([
            np.ascontiguousarray(
                xT[:, :, jt * TQ:(jt + 1) * TQ].transpose(1, 0, 2)
                .reshape(128, KD * TQ))
            for jt in range(njq)])
        wq_c = wq16[:, hg * DQ:(hg + 1) * DQ]
        in_maps.append({
            "xTt": xtt,
            "wqa": _pmajor(wq_c[:, 0:DQ // 2]),
            "wqb": _pmajor(wq_c[:, DQ // 2:DQ]),
            "wk": _pmajor(wk16[:, hg * DKV:(hg + 1) * DKV]),
            "wv": _pmajor(wv16[:, hg * DKV:(hg + 1) * DKV]),
            "wo": _pmajor(wo16[hg * DQ:(hg + 1) * DQ, :]),
            "cosT": cos, "sinT": sin_s, "tri": tri,
        })
    return in_maps


_NC_CACHE = {}


def get_nc(T=2048):
    if T not in _NC_CACHE:
        _NC_CACHE[T] = build_nc(T)
    return _NC_CACHE[T]


def run(inputs, trace=False, **kw):
    """Returns (full_output [B,T,D] f32, BassKernelResults)."""
    from concourse import bass_utils
    x = np.asarray(inputs["x"], dtype=np.float32)
    T = x.shape[1]
    nc = get_nc(T)
    in_maps = build_in_maps(x, inputs["wq"], inputs["wk"], inputs["wv"],
                            inputs["wo"], T)
    res = bass_utils.run_bass_kernel_spmd(nc, in_maps,
                                          core_ids=list(range(NCORES)),
                                          trace=trace, **kw)
    outs = [np.asarray(r["out"]) for r in res.results]
    full = np.empty((B, T, D), dtype=np.float32)
    for b in range(B):
        full[b] = outs[2 * b] + outs[2 * b + 1]
    return full, res


def kernel(x, mask, wq, wk, wv, wo):
    full, _ = run({"x": x, "mask": mask, "wq": wq, "wk": wk, "wv": wv, "wo": wo})
    return full
